# revision 1
# baseline (speedup 1.0000x reference)
"""Trainium2 Bass kernel for nn_MultiHeadAttention_45672682226228.

The reference module computes multi-head attention but everything except the
V projection is dead code (DCE'd under jit): the returned value is

    out[b, s, 64*h + q] = x[b, s, 768 + 64*h + q]
                        + sum_d x[b, s, 256*h + d] * W_v[q, d]

i.e. a per-token block-diagonal matmul (4 heads x [256 -> 64]) plus a
residual add of the last head's input slice.  W_q / W_k are unused.

Sharding: data-parallel over batch B=16 -> 2 batches (8192 tokens) per core
across 8 NeuronCores.  Per core:

  x_shard [8192, 1024] fp32  ->  out [8192, 256] fp32

On-chip dataflow per 512-token group (16 groups):
  1. DMA x tile [128p, 4s, 1024] (token-major).
  2. TensorE transposes (fp32r, 128x128) -> PSUM [d, t] chunks.
  3. DVE/ACT copy PSUM -> SBUF xT [128d, 8j, 512t].
  4. TensorE matmuls: out.T[c-chunk, t] += Wblk_j.T @ xT_j (fp32r, N=512),
     4 accumulating matmuls per 128-wide c-chunk.
  5. copy PSUM -> SBUF out.T, TensorE transpose back -> PSUM [t, c].
  6. DVE adds residual x[:, 768:1024] and writes SBUF -> DMA out.
"""

import os
import numpy as np

P = 128
TPC = 8192          # tokens per core
NCORES = 8
GROUPS = 16         # 512-token groups per core
SUBT = 4            # 128-token subtiles per group

_STATE = {}


def _pack_wblk(W_v: np.ndarray) -> np.ndarray:
    """Pack W_v [64, 256] into per-d-chunk stationary blocks [128, 8, 128].

    wblk[dd, j, col]: d-chunk j covers global d in [128j, 128j+128);
    head h = j//2, half = j%2.  Within c-chunk cc = j//4 the head's 64
    output cols sit at offset 64*(h%2).  Zeros elsewhere.
    """
    W_v = np.asarray(W_v, np.float32)
    wblk = np.zeros((P, 8, P), np.float32)
    for j in range(8):
        h, half = j // 2, j % 2
        c0 = 64 * (h % 2)
        wblk[:, j, c0:c0 + 64] = W_v[:, 128 * half:128 * half + 128].T
    return wblk


def _build_nc(tpc=TPC):
    from contextlib import ExitStack

    import concourse.mybir as mybir
    import concourse.tile as tile
    from concourse import bacc
    from concourse.bass import ts

    f32 = mybir.dt.float32
    f32r = mybir.dt.float32r
    groups = tpc // 512

    nc = bacc.Bacc("TRN2", target_bir_lowering=False, debug=False)
    x_h = nc.dram_tensor("x", [tpc, 1024], f32r, kind="ExternalInput")
    w_h = nc.dram_tensor("wblk", [P, 8, P], f32r, kind="ExternalInput")
    i_h = nc.dram_tensor("ident", [P, P], f32r, kind="ExternalInput")
    o_h = nc.dram_tensor("out", [tpc, 256], f32, kind="ExternalOutput")

    xg = x_h.rearrange("(g s p) d -> g p s d", p=P, s=SUBT)
    og = o_h.rearrange("(g s p) c -> g p s c", p=P, s=SUBT)

    with ExitStack() as ctx:
        tc = ctx.enter_context(tile.TileContext(nc))
        const = ctx.enter_context(tc.tile_pool(name="const", bufs=1))
        xin = ctx.enter_context(tc.tile_pool(name="xin", bufs=6))
        xtp = ctx.enter_context(tc.tile_pool(name="xtp", bufs=3))
        otp = ctx.enter_context(tc.tile_pool(name="otp", bufs=3))
        osb = ctx.enter_context(tc.tile_pool(name="osb", bufs=3))
        ps_xt = ctx.enter_context(tc.tile_pool(name="ps_xt", bufs=4, space="PSUM"))
        ps_mm = ctx.enter_context(tc.tile_pool(name="ps_mm", bufs=2, space="PSUM"))
        ps_fin = ctx.enter_context(tc.tile_pool(name="ps_fin", bufs=2, space="PSUM"))

        identr = const.tile([P, P], f32r)
        nc.sync.dma_start(identr[:], i_h[:])

        w_sb = const.tile([P, 8, P], f32r)
        nc.sync.dma_start(w_sb[:], w_h[:])

        # software-pipelined with a two-stage skew: transposes of group g,
        # matmuls of group g-1, output phase of group g-2 — the PE always
        # has independent work while PSUM->SBUF copies drain.
        x_tiles = {}
        xt_tiles = {}
        ot_tiles = {}

        def stage_load(g):
            if g == 0 or g >= groups:
                return  # group 0 is loaded inside stage_transpose (fast start)
            x_sb = xin.tile([P, SUBT, 1024], f32r)
            # alternate the two HWDGE rings (SP / ACT) so neither descriptor
            # FIFO backs up behind a burst of queued loads
            eng = nc.sync if g % 2 == 0 else nc.scalar
            eng.dma_start(x_sb[:], xg[g])
            x_tiles[g] = x_sb

        def stage_transpose(g):
            xt_sb = xtp.tile([P, 8, 512], f32r)
            if g == 0:
                x_sb = xin.tile([P, SUBT, 1024], f32r)
                # fast start: load group 0 subtile-by-subtile and transpose
                # s-major so the PE starts as soon as subtile 0 lands
                xsub = xg[g]  # [128, 4, 1024]
                for s in range(SUBT):
                    nc.sync.dma_start(x_sb[:, s, :], xsub[:, s, :])
                    for half in range(2):
                        pt = ps_xt.tile([P, 512], f32r)
                        for jj in range(4):
                            j = half * 4 + jj
                            nc.tensor.transpose(
                                pt[:, ts(jj, P)],
                                x_sb[:, s, ts(j, P)],
                                identr[:],
                            )
                        src = pt[:].rearrange("p (j t) -> p j t", j=4)
                        dst = xt_sb[:, half * 4:half * 4 + 4, ts(s, P)]
                        if half == 0:
                            nc.vector.tensor_copy(dst, src)
                        else:
                            nc.scalar.copy(dst, src)
                x_tiles[g] = x_sb
                xt_tiles[g] = xt_sb
                return
            x_sb = x_tiles[g]
            for j in range(8):
                pt = ps_xt.tile([P, 512], f32r)
                for s in range(SUBT):
                    nc.tensor.transpose(
                        pt[:, ts(s, P)],
                        x_sb[:, s, ts(j, P)],
                        identr[:],
                    )
                if j % 8 < 3:
                    nc.vector.tensor_copy(xt_sb[:, j, :], pt[:])
                else:
                    nc.scalar.copy(xt_sb[:, j, :], pt[:])
            xt_tiles[g] = xt_sb

        def stage_mm(g):
            xt_sb = xt_tiles.pop(g)
            # V projection: out.T[c, t] in two 128-wide c-chunks
            ot_sb = otp.tile([P, 2, 512], f32r)
            for cc in range(2):
                pm = ps_mm.tile([P, 512], f32)
                for i, j in enumerate(range(4 * cc, 4 * cc + 4)):
                    nc.tensor.matmul(
                        pm[:],
                        w_sb[:, j, :],
                        xt_sb[:, j, :],
                        start=(i == 0),
                        stop=(i == 3),
                    )
                nc.scalar.copy(ot_sb[:, cc, :], pm[:])
            ot_tiles[g] = ot_sb

        def stage_out(g):
            x_sb = x_tiles.pop(g)
            ot_sb = ot_tiles.pop(g)
            # transpose back to [t, c] and add residual
            o_sb = osb.tile([P, SUBT, 256], f32)
            last = g >= groups - 2
            for s in range(SUBT):
                pf = ps_fin.tile([P, 256], f32r)
                for cc in range(2):
                    nc.tensor.transpose(
                        pf[:, ts(cc, P)],
                        ot_sb[:, cc, ts(s, P)],
                        identr[:],
                    )
                nc.vector.tensor_add(
                    o_sb[:, s, :],
                    pf[:].bitcast(f32),
                    x_sb[:, s, 768:1024].bitcast(f32),
                )
                if last:
                    # shrink the kernel tail: ship each subtile as soon as
                    # its residual add completes; the input stream is done
                    # by now so the low-latency Sync HWDGE ring is free
                    nc.sync.dma_start(og[g][:, s, :], o_sb[:, s, :])
            if not last:
                # SWDGE (GpSimd) so output stores don't head-of-line block
                # the input loads on the Sync HWDGE ring
                nc.gpsimd.dma_start(og[g], o_sb[:])

        for g in range(groups + 1):
            if g == 0:
                stage_transpose(0)   # includes group 0's loads
                stage_load(1)
                stage_load(2)
                continue
            if g + 2 < groups:
                stage_load(g + 2)
            if g < groups:
                stage_transpose(g)
            stage_mm(g - 1)
            if g - 2 >= 0:
                stage_out(g - 2)
            if g == groups:
                stage_out(g - 1)     # compressed tail

    nc.compile()
    return nc


def _install_ntff_hook():
    """Provide antenv.axon_hooks (absent in this image) so trace=True works.

    Reconstructs the hook trn_boot would have registered at agent boot.
    """
    import sys
    import types

    if "antenv.axon_hooks" in sys.modules:
        return
    try:
        import trn_agent_boot.trn_boot as tb

        hook = tb._ntff_profile_via_ctypes("/opt/axon/libaxon_pjrt.so")
    except Exception:
        hook = None
    mod = types.ModuleType("antenv.axon_hooks")
    mod.get_axon_ntff_profile_hook = lambda: hook
    mod.set_axon_ntff_profile_hook = lambda h: None
    sys.modules["antenv.axon_hooks"] = mod
    try:
        import antenv

        antenv.axon_hooks = mod
    except ImportError:
        pass


def kernel(x, W_q=None, W_k=None, W_v=None, **_):
    from concourse.bass_utils import run_bass_kernel_spmd

    if "nc" not in _STATE:
        _STATE["nc"] = _build_nc()
    nc = _STATE["nc"]

    x = np.asarray(x, np.float32)
    b, s, e = x.shape
    xf = np.ascontiguousarray(x.reshape(b * s, e))
    wblk = _pack_wblk(W_v)

    ident = np.eye(P, dtype=np.float32)
    in_maps = [
        {"x": xf[c * TPC:(c + 1) * TPC], "wblk": wblk, "ident": ident}
        for c in range(NCORES)
    ]
    trace = os.environ.get("KERNEL_TRACE", "0") == "1"
    if trace:
        _install_ntff_hook()
    res = run_bass_kernel_spmd(nc, in_maps, core_ids=list(range(NCORES)), trace=trace)
    _STATE["last_results"] = res
    out = np.concatenate([r["out"] for r in res.results], axis=0)
    return out.reshape(b, s, 256)



# revision 2
# speedup vs baseline: 1.8374x; 1.8374x over previous
"""Trainium2 Bass kernel for nn_MultiHeadAttention_45672682226228.

The reference module computes multi-head attention but everything except the
V projection is dead code (DCE'd under jit): the returned value is

    out[b, s, 64*h + q] = x[b, s, 768 + 64*h + q]
                        + sum_d x[b, s, 256*h + d] * W_v[q, d]

i.e. a per-token block-diagonal matmul (4 heads x [256 -> 64]) plus a
residual add of the last head's input slice.  W_q / W_k are unused.

Kernel strategy (v2):
  * Data-parallel over batch B=16 -> 2 batches (8192 tokens) per core.
  * The residual is folded into the weights: W_big [1024, 256] =
    blockdiag(W_v.T per head) + rows 768:1024 get +I.  The whole module is
    then a single matmul  out = x @ W_big.
  * x is pre-transposed and cast to bf16 on the HOST, so the device streams
    xT [1024, 8192] tiles straight into accumulating PE matmuls
    outT[c,t] = sum_j W_big[j].T @ xT[j] - no on-chip transposes at all.
  * Per 512-token group: c-chunk 0 needs d-chunks {0,1,2,3,6}, c-chunk 1
    needs {4,5,6,7} (W_big is block-sparse) -> 9 matmuls of N=512.
  * Output is computed as outT [256, 8192] f32 in PSUM, cast to bf16 in
    SBUF (halves store traffic), DMA'd out, and un-transposed/upcast on the
    host.  bf16 end-to-end error ~5e-3, well inside the 2e-2 gate.

Per-core HBM traffic: 16 MiB in + 4 MiB out (vs 40 MiB for the f32
transpose-on-device kernel).
"""

import os
import numpy as np

P = 128
TPC = 8192          # tokens per core
NCORES = 8
TBLK = 2048         # tokens per DMA tile
NTB = TPC // TBLK   # 4
GRP = 512           # tokens per matmul group (PSUM bank = 512 f32)
NGRP = TBLK // GRP  # 4

# d-chunks feeding each 128-wide c-chunk of W_big (block sparsity)
CC_J = [[0, 1, 2, 3, 6], [4, 5, 6, 7]]
# load order per t-block: c-chunk 0's inputs first
LOAD_ORDER = [0, 1, 6, 2, 3, 4, 5, 7]

_STATE = {}


def _bf16():
    import ml_dtypes

    return ml_dtypes.bfloat16


def _pack_wbig(W_v: np.ndarray) -> np.ndarray:
    """W_big [1024, 256] = blockdiag(W_v.T) + I on rows 768:1024.

    Packed as [p, j, c] (d-within-chunk, d-chunk, out-col) so the DMA into
    SBUF [128, 8, 256] is fully contiguous.
    """
    W_v = np.asarray(W_v, np.float32)
    Wb = np.zeros((1024, 256), np.float32)
    for h in range(4):
        Wb[256 * h:256 * (h + 1), 64 * h:64 * (h + 1)] = W_v.T
    Wb[np.arange(768, 1024), np.arange(256)] += 1.0
    pj = Wb.reshape(8, P, 256).transpose(1, 0, 2)
    return np.ascontiguousarray(pj).astype(_bf16())


def _build_nc(tpc=TPC):
    from contextlib import ExitStack

    import concourse.mybir as mybir
    import concourse.tile as tile
    from concourse import bacc
    from concourse.bass import ds, ts

    bf16 = mybir.dt.bfloat16
    f32 = mybir.dt.float32

    nc = bacc.Bacc("TRN2", target_bir_lowering=False, debug=False)
    xt_h = nc.dram_tensor("xt", [8, P, tpc], bf16, kind="ExternalInput")
    w_h = nc.dram_tensor("wbig", [P, 8, 256], bf16, kind="ExternalInput")
    o_h = nc.dram_tensor("out", [2, P, tpc], bf16, kind="ExternalOutput")

    ntb = tpc // TBLK

    with ExitStack() as ctx:
        tc = ctx.enter_context(tile.TileContext(nc))
        sb = ctx.enter_context(tc.tile_pool(name="sb", bufs=1))
        ps = ctx.enter_context(tc.tile_pool(name="ps", bufs=4, space="PSUM"))

        w_sb = sb.tile([P, 8, 256], bf16)
        nc.sync.dma_start(w_sb[:], w_h[:])

        xt_sb = sb.tile([P, 8, tpc], bf16)   # 128 KiB / partition
        out_sb = sb.tile([P, 2, tpc], bf16)  # 32 KiB / partition

        # Enqueue every input load up-front; the two HWDGE rings stream them
        # back-to-back at full rate while the PE chews through groups.
        for tb in range(ntb):
            for i, j in enumerate(LOAD_ORDER):
                eng = nc.scalar if (tb * 8 + i) % 2 == 0 else nc.sync
                eng.dma_start(
                    xt_sb[:, j, ts(tb, TBLK)], xt_h[j, :, ts(tb, TBLK)]
                )

        for tb in range(ntb):
            for g in range(NGRP):
                tsl = ds(tb * TBLK + g * GRP, GRP)
                for cc in range(2):
                    pm = ps.tile([P, GRP], f32, tag=f"pm{cc}", name=f"pm{cc}")
                    js = CC_J[cc]
                    for i, j in enumerate(js):
                        nc.tensor.matmul(
                            pm[:],
                            w_sb[:, j, ts(cc, P)],
                            xt_sb[:, j, tsl],
                            start=(i == 0),
                            stop=(i == len(js) - 1),
                        )
                    # cast f32 PSUM -> bf16 SBUF; keep DVE free of DMA work
                    if cc == 0:
                        nc.vector.tensor_copy(out_sb[:, 0, tsl], pm[:])
                    else:
                        nc.scalar.copy(out_sb[:, 1, tsl], pm[:])
            if tb < ntb - 1:
                # SWDGE so stores don't head-of-line block input HWDGE rings
                for cc in range(2):
                    nc.gpsimd.dma_start(
                        o_h[cc, :, ts(tb, TBLK)], out_sb[:, cc, ts(tb, TBLK)]
                    )
            else:
                # input rings are empty by now: ship the tail per-group on
                # the low-latency HWDGE rings to shrink the kernel tail
                for g in range(NGRP):
                    tsl = ds(tb * TBLK + g * GRP, GRP)
                    for cc in range(2):
                        eng = nc.sync if cc == 0 else nc.scalar
                        eng.dma_start(o_h[cc, :, tsl], out_sb[:, cc, tsl])

    nc.compile()
    return nc


def _install_ntff_hook():
    """Provide antenv.axon_hooks (absent in this image) so trace=True works."""
    import sys
    import types

    if "antenv.axon_hooks" in sys.modules:
        return
    try:
        import trn_agent_boot.trn_boot as tb

        hook = tb._ntff_profile_via_ctypes("/opt/axon/libaxon_pjrt.so")
    except Exception:
        hook = None
    mod = types.ModuleType("antenv.axon_hooks")
    mod.get_axon_ntff_profile_hook = lambda: hook
    mod.set_axon_ntff_profile_hook = lambda h: None
    sys.modules["antenv.axon_hooks"] = mod
    try:
        import antenv

        antenv.axon_hooks = mod
    except ImportError:
        pass


def kernel(x, W_q=None, W_k=None, W_v=None, **_):
    from concourse.bass_utils import run_bass_kernel_spmd

    if "nc" not in _STATE:
        _STATE["nc"] = _build_nc()
    nc = _STATE["nc"]
    bf16 = _bf16()

    x = np.asarray(x, np.float32)
    b, s, e = x.shape
    xf = x.reshape(b * s, e).astype(bf16)  # one contiguous f32->bf16 pass
    wbig = _pack_wbig(W_v)

    in_maps = []
    for c in range(NCORES):
        xtc = np.ascontiguousarray(xf[c * TPC:(c + 1) * TPC].T)  # [1024, TPC]
        in_maps.append({"xt": xtc.reshape(8, P, TPC), "wbig": wbig})

    trace = os.environ.get("KERNEL_TRACE", "0") == "1"
    if trace:
        _install_ntff_hook()
    res = run_bass_kernel_spmd(nc, in_maps, core_ids=list(range(NCORES)), trace=trace)
    _STATE["last_results"] = res

    outs = []
    for r in res.results:
        oc = np.asarray(r["out"]).reshape(256, TPC)  # [c, t] bf16
        outs.append(oc.T.astype(np.float32))         # [t, c] f32
    out = np.concatenate(outs, axis=0)
    return out.reshape(b, s, 256)
